# revision 31
# baseline (speedup 1.0000x reference)
"""Trainium2 Bass kernel for nn_DecoderAttentionLSTM (v2).

Data-parallel over 8 NeuronCores on the batch axis (BL=8 batches/core).
Per core, the 256-step decode scan runs locally with all weights and the
precomputed h_proj = h @ We1[:U] SBUF-resident in bf16; h streams from
DRAM each step (context matmul only).

v2 structure (vs v1):
  - All thin-M (M=8) matmuls are 4x column-tiled: 4 concurrent rhs
    streams into distinct 32-column PE groups (outputs at PSUM partition
    bases 0/32/64/96), quartering weight-streaming time.
  - The 4 col-group partials are summed AND transposed in one PE
    "combine" matmul per 128-col chunk: out[feat,b] = C_chunk.T @ E,
    where E[32j+b, b] = 1. This yields feature-major [128, b] layouts
    for y1/sproj/gates/c, so activations get per-partition ACT bias and
    the LSTM cell runs on [128, 64] tiles (16x fewer DVE cycles than
    [8, 1024]), and the state needs no final transpose.
  - Attention z-add / sigmoid / e-dot pipelined with the gate matmuls.
  - softmax exp() via degree-4 polynomial (sigmoid output in (0,1)), so
    only the Sigmoid/Tanh ACT table set is ever loaded.
"""

import sys

sys.path.insert(0, "/opt/trn_rl_repo")

from contextlib import ExitStack  # noqa: E402

import ml_dtypes  # noqa: E402
import numpy as np  # noqa: E402

import concourse.bass as bass  # noqa: E402
import concourse.mybir as mybir  # noqa: E402
import concourse.tile as tile  # noqa: E402
from concourse import bacc  # noqa: E402
from concourse.bass import ds, ts  # noqa: E402
from concourse.bass_utils import run_bass_kernel_spmd  # noqa: E402
from concourse.masks import make_identity  # noqa: E402

B, S, U, T = 64, 256, 1024, 512
NCORES = 8
BL = B // NCORES          # 8 local batches
UC = U // 128             # 8 u-chunks
TC4 = (T + U) // 128      # 12 k-chunks for the gate matmuls
G = 4 * U                 # 4096 gate outputs (i|f|o|g)
BS = BL * S               # 2048

bf16 = mybir.dt.bfloat16
f32 = mybir.dt.float32
AF = mybir.ActivationFunctionType
ALU = mybir.AluOpType

# degree-4 polynomial for exp(x) on [0, 1] (abs err ~ 3e-6, values >= 1)
_x = np.linspace(0.0, 1.0, 2001)
_EXP_C = np.polyfit(_x, np.exp(_x), 4)[::-1]  # c0..c4


def _mm(nc, out, lhsT, rhs, start, stop, tile_position=None):
    nc.tensor.matmul(out, lhsT, rhs, start=start, stop=stop,
                     skip_group_check=True, tile_position=tile_position)


def build(nsteps=S, unroll=8, static_loop=False):
    """Build the Bass module (same program for all 8 cores)."""
    nc = bacc.Bacc("TRN2", target_bir_lowering=False, debug=False)

    # ---- DRAM I/O (per-core shapes; wrapper does layout/casts in numpy)
    d_hbf = nc.dram_tensor("h_bf", [2 * BL, 128, U], bf16, kind="ExternalInput")
    d_hT = nc.dram_tensor("hT_bf", [UC, 128, BS], bf16, kind="ExternalInput")
    d_we1h = nc.dram_tensor("We1h", [UC, 128, U], bf16, kind="ExternalInput")
    d_wsy = nc.dram_tensor("Wsy", [UC, 128, 2 * U], bf16, kind="ExternalInput")
    d_wy2 = nc.dram_tensor("Wy2b", [UC, 128, T], bf16, kind="ExternalInput")
    d_w4 = nc.dram_tensor("W4", [TC4, 128, G], bf16, kind="ExternalInput")
    d_we2 = nc.dram_tensor("We2c", [128, UC], bf16, kind="ExternalInput")
    d_E = nc.dram_tensor("Emat", [128, BL], bf16, kind="ExternalInput")
    d_ones1 = nc.dram_tensor("ones1", [1, BL], bf16, kind="ExternalInput")
    d_by2row = nc.dram_tensor("by2row", [1, T], bf16, kind="ExternalInput")
    d_by1b = nc.dram_tensor("by1b", [128, UC * BL], f32, kind="ExternalInput")
    d_be1b = nc.dram_tensor("be1b", [128, UC * BL], f32, kind="ExternalInput")
    d_b4b = nc.dram_tensor("b4b", [128, 4 * UC * BL], bf16, kind="ExternalInput")
    d_be2r = nc.dram_tensor("be2r", [BL, 1], f32, kind="ExternalInput")
    d_s0 = nc.dram_tensor("s0b", [BL, U], bf16, kind="ExternalInput")
    d_out = nc.dram_tensor("ys", [BL, S * T], f32, kind="ExternalOutput")

    with tile.TileContext(nc) as tc, ExitStack() as ctx:
        # ================= static SBUF (persists for the whole kernel)
        st = ctx.enter_context(tc.tile_pool(name="static", bufs=1))
        wsy_sb = [st.tile([128, 2 * U], bf16, tag=f"wsy{k}", name=f"wsy{k}") for k in range(UC)]
        wy2_sb = [st.tile([128, T], bf16, tag=f"wy2{k}", name=f"wy2{k}") for k in range(UC)]
        w4_sb = [st.tile([128, G], bf16, tag=f"w4{k}", name=f"w4{k}") for k in range(TC4)]
        hproj_sb = [st.tile([128, BS], bf16, tag=f"hp{k}", name=f"hp{k}") for k in range(UC)]
        we2d_sb = [st.tile([128, 8 * BL], bf16, tag=f"we2d{k}", name=f"we2d{k}") for k in range(UC)]
        E_sb = st.tile([128, BL], bf16, tag="Emat")
        ones1_sb = st.tile([1, BL], bf16, tag="ones1")
        by2row_sb = st.tile([1, T], bf16, tag="by2row")
        by1b_sb = st.tile([128, UC * BL], f32, tag="by1b")
        be1b_sb = st.tile([128, UC * BL], f32, tag="be1b")
        b4b_sb = st.tile([128, 4 * UC * BL], bf16, tag="b4b")
        be2r_sb = st.tile([BL, 1], f32, tag="be2r")
        id8 = st.tile([8, 8], bf16, tag="id8")
        A_ld = st.tile([128, 128], bf16, tag="A_ld")
        sT = [st.tile([128, UC * BL], bf16, tag=f"sT{p}", name=f"sT{p}") for p in range(2)]
        y1pre = st.tile([128, UC * BL], f32, tag="y1pre")
        y1t_sb = st.tile([128, UC * BL], bf16, tag="y1t")
        sprojT_sb = st.tile([128, UC * BL], f32, tag="sprojT")
        xhy_sb = st.tile([128, 4 * BL], bf16, tag="xhy")
        y_sb = st.tile([BL, T], f32, tag="y_sb")
        gact = st.tile([128, 4 * UC * BL], bf16, tag="gact")
        esig = st.tile([BL, S], f32, tag="esig")
        er = st.tile([BL, S], f32, tag="er")
        eq = st.tile([BL, S], f32, tag="eq")
        ea_bf = st.tile([BL, S], bf16, tag="ea_bf")
        den = st.tile([BL, 1], f32, tag="den")
        rden = st.tile([BL, 1], f32, tag="rden")
        gsum = st.tile([128, 4 * UC * BL], bf16, tag="gsum")
        t1f = st.tile([128, UC * BL], f32, tag="t1f")
        t2f = st.tile([128, UC * BL], f32, tag="t2f")
        cnew = st.tile([128, UC * BL], f32, tag="cnew")
        thb = st.tile([128, UC * BL], bf16, tag="thb")

        # ================= init: load weights, build masks
        make_identity(nc, id8[:])
        nc.vector.memset(A_ld[:], 0.0)
        for k in range(UC):
            nc.sync.dma_start(wsy_sb[k][:], d_wsy[k])
            nc.sync.dma_start(wy2_sb[k][:], d_wy2[k])
        for k in range(TC4):
            nc.sync.dma_start(w4_sb[k][:], d_w4[k])
        nc.sync.dma_start(E_sb[:], d_E[:])
        nc.sync.dma_start(ones1_sb[:], d_ones1[:])
        nc.sync.dma_start(by2row_sb[:], d_by2row[:])
        nc.sync.dma_start(by1b_sb[:], d_by1b[:])
        nc.sync.dma_start(be1b_sb[:], d_be1b[:])
        nc.sync.dma_start(b4b_sb[:], d_b4b[:])
        nc.sync.dma_start(be2r_sb[:], d_be2r[:])

        with tc.tile_pool(name="initp", bufs=2) as initp:
            we2_stage = initp.tile([128, UC], bf16, tag="we2stage")
            nc.sync.dma_start(we2_stage[:], d_we2[:])
            # We2 block-diagonal lhsT tiles: we2d[uc][:, 9*b] = We2 chunk uc
            for k in range(UC):
                nc.vector.memset(we2d_sb[k][:], 0.0)
                for b in range(BL):
                    nc.vector.tensor_copy(
                        we2d_sb[k][:, 9 * b : 9 * b + 1], we2_stage[:, k : k + 1]
                    )
            # initial state: s0 -> sT[0]
            s_bf = initp.tile([BL, U], bf16, tag="s_bf")
            nc.sync.dma_start(s_bf[:], d_s0[:])
            with tc.tile_pool(name="ps_init", bufs=1, space="PSUM") as ps_init:
                psT0 = ps_init.tile([128, UC * BL], bf16, tag="tr0")
                for q in range(UC):
                    nc.tensor.transpose(
                        psT0[:, 8 * q : 8 * q + 8],
                        s_bf[:, 128 * q : 128 * (q + 1)], id8[:]
                    )
                nc.vector.tensor_copy(sT[0][:], psT0[:])

        # ================= h_proj = (h @ We1[:U])^T into SBUF-resident tiles
        with tc.tile_pool(name="hp_w", bufs=3) as hp_w, \
             tc.tile_pool(name="hp_r", bufs=3) as hp_r, \
             tc.tile_pool(name="hp_ps", bufs=2, space="PSUM") as hp_ps:
            for m in range(UC):
                for n in range(BS // 512):
                    ps = hp_ps.tile([128, 512], f32, tag="hp_ps", name="hp_ps")
                    for k in range(UC):
                        wt = hp_w.tile([128, 128], bf16, tag="hp_w", name="hp_w")
                        nc.sync.dma_start(wt[:], d_we1h[k, :, 128 * m : 128 * (m + 1)])
                        rt = hp_r.tile([128, 512], bf16, tag="hp_r", name="hp_r")
                        nc.sync.dma_start(rt[:], d_hT[k, :, 512 * n : 512 * (n + 1)])
                        _mm(nc, ps[:], wt[:], rt[:],
                            start=(k == 0), stop=(k == UC - 1))
                    nc.vector.tensor_copy(
                        hproj_sb[m][:, 512 * n : 512 * (n + 1)], ps[:])

        # ================= working pools for the scan
        # PSUM budget (8 banks): ps_a 2x[128,1024]f32 = 4, ps_b 2x(<=1) = 2,
        # ps_e 1, ps_g 1.
        ps_a = ctx.enter_context(tc.tile_pool(name="ps_a", bufs=2, space="PSUM"))
        ps_b = ctx.enter_context(tc.tile_pool(name="ps_b", bufs=2, space="PSUM"))
        ps_e = ctx.enter_context(tc.tile_pool(name="ps_e", bufs=1, space="PSUM"))
        ps_g = ctx.enter_context(tc.tile_pool(name="ps_g", bufs=1, space="PSUM"))
        z_pool = ctx.enter_context(tc.tile_pool(name="z_pool", bufs=4))
        h_pool = ctx.enter_context(tc.tile_pool(name="h_pool", bufs=4))
        c_pool = ctx.enter_context(tc.tile_pool(name="c_pool", bufs=2))

        def col_mm(out_ps, j, lhsT, rhs, start, stop):
            """Matmul into column-group j (PSUM partitions 32j..32j+8)."""
            _mm(nc, out_ps[32 * j : 32 * j + 8, :], lhsT, rhs, start, stop,
                tile_position=(0, 32 * j))

        def combine(src_sb, c0, n, out_ps, o0):
            """out_ps[:, o0+8c] = src_sb[:, 128c].T @ E  (sum 4 groups +
            transpose) for n consecutive 128-col chunks."""
            for c in range(n):
                _mm(nc, out_ps[:, o0 + 8 * c : o0 + 8 * c + 8],
                    src_sb[:, 128 * (c0 + c) : 128 * (c0 + c + 1)], E_sb[:],
                    start=True, stop=True)

        def step_body(step_ap, j):
            """One decode step. step_ap: dynamic step index (ScalarValue)."""
            rd = sT[j % 2]
            wr = sT[(j + 1) % 2]

            e1_tiles = []
            e_ps = ps_e.tile([128, S], f32, tag="e")
            edone = [0]

            def produce(uc):
                """z = hproj[uc] + sproj (DVE + 2 blocks on GPSIMD),
                e1 = sigmoid(z) (ACT)."""
                z_t = z_pool.tile([128, BS], bf16, tag="z", name="z_t")
                for b in range(BL):
                    eng = nc.gpsimd if b >= 6 else nc.vector
                    eng.tensor_scalar_add(
                        z_t[:, 256 * b : 256 * (b + 1)],
                        hproj_sb[uc][:, 256 * b : 256 * (b + 1)],
                        sprojT_sb[:, 8 * uc + b : 8 * uc + b + 1])
                e1_t = z_pool.tile([128, BS], bf16, tag="z", name="e1_t")
                nc.scalar.activation(e1_t[:], z_t[:], AF.Sigmoid)
                e1_tiles.append(e1_t)

            def edot_mm(uc, b):
                g = uc % 4
                _mm(nc, e_ps[32 * g : 32 * g + 8, :],
                    we2d_sb[uc][:, 8 * b : 8 * b + 8],
                    e1_tiles[uc][:, 256 * b : 256 * (b + 1)],
                    start=(uc < 4 and b == 0),
                    stop=(uc >= 4 and b == BL - 1),
                    tile_position=(0, 32 * g))

            def drain_edots(upto):
                """e[b,:] += We2[uc] . e1[uc][:, b-block]; column-group uc%4.
                Interleave batches across uc pairs so adjacent matmuls hit
                different column groups."""
                while edone[0] < upto:
                    e0 = edone[0]
                    if upto - e0 >= 2:
                        for b in range(BL):
                            edot_mm(e0, b)
                            edot_mm(e0 + 1, b)
                        edone[0] += 2
                    else:
                        for b in range(BL):
                            edot_mm(e0, b)
                        edone[0] += 1

            # ---- h prefetch for the context matmul (consumed at step end).
            # Issue DMAs in the ctx consumption order (group-interleaved).
            h_tiles = {}
            for p in range(4):
                for g4 in range(4):
                    ci = 4 * g4 + p
                    h_t = h_pool.tile([128, U], bf16, tag="h", name="h_t")
                    nc.gpsimd.dma_start(h_t[:], d_hbf[ci])
                    h_tiles[ci] = h_t

            # ---- 1) sproj = s @ We1_s (col-tiled), combine+transpose
            sp_ps = ps_a.tile([128, U], f32, tag="mm", name="sp_ps")
            for r in range(2):
                for hh in range(2):
                    for g4 in range(4):
                        k = 4 * r + g4
                        _mm(nc, sp_ps[32 * g4 : 32 * g4 + 8, 512 * hh : 512 * (hh + 1)],
                            rd[:, 8 * k : 8 * k + 8],
                            wsy_sb[k][:, U + 512 * hh : U + 512 * (hh + 1)],
                            start=(r == 0), stop=(r == 1),
                            tile_position=(0, 32 * g4))
            C1 = c_pool.tile([128, U], bf16, tag="c", name="C1")
            nc.vector.tensor_copy(C1[:], sp_ps[:])
            pT1 = ps_b.tile([128, UC * BL], f32, tag="cmb", name="pT1")
            combine(C1, 0, UC, pT1, 0)
            nc.vector.tensor_add(sprojT_sb[:], pT1[:], be1b_sb[:])

            # ---- start attention produce early (ucs 0..1)
            produce(0)
            produce(1)

            # ---- 2) y1 = s @ Wy1 (col-tiled), combine, tanh
            y1_ps = ps_a.tile([128, U], f32, tag="mm", name="y1_ps")
            for r in range(2):
                for hh in range(2):
                    for g4 in range(4):
                        k = 4 * r + g4
                        _mm(nc, y1_ps[32 * g4 : 32 * g4 + 8, 512 * hh : 512 * (hh + 1)],
                            rd[:, 8 * k : 8 * k + 8],
                            wsy_sb[k][:, 512 * hh : 512 * (hh + 1)],
                            start=(r == 0), stop=(r == 1),
                            tile_position=(0, 32 * g4))
            C0 = c_pool.tile([128, U], bf16, tag="c", name="C0")
            nc.vector.tensor_copy(C0[:], y1_ps[:])
            pT0 = ps_b.tile([128, UC * BL], f32, tag="cmb", name="pT0")
            combine(C0, 0, UC, pT0, 0)
            nc.vector.tensor_add(y1pre[:], pT0[:], by1b_sb[:])
            nc.scalar.activation(y1t_sb[:], y1pre[:], AF.Tanh)

            # ---- 3) y = y1 @ Wy2 + by2 (col-tiled + bias-in-stream)
            y_ps = ps_a.tile([128, T], f32, tag="mm", name="y_ps")
            for r in range(2):
                for g4 in range(4):
                    k = 4 * r + g4
                    col_mm(y_ps, g4, y1t_sb[:, 8 * k : 8 * k + 8],
                           wy2_sb[k][:], start=(r == 0),
                           stop=(r == 1 and g4 > 0))
            # bias row into group 0 (counted once across the group sum)
            _mm(nc, y_ps[0:8, :], ones1_sb[:], by2row_sb[:],
                start=False, stop=True)
            C_y = c_pool.tile([128, T], bf16, tag="c", name="C_y")
            nc.vector.tensor_copy(C_y[:], y_ps[:])
            # row-major y for the output DMA
            y2_ps = ps_b.tile([BL, T], f32, tag="cmb", name="y2_ps")
            _mm(nc, y2_ps[:], E_sb[:], C_y[:], start=True, stop=True)
            nc.scalar.copy(y_sb[:], y2_ps[:])
            nc.sync.dma_start(d_out[:, ts(step_ap, T)], y_sb[:])
            # transposed y chunks -> xhy (gate lhsT)
            pxh = ps_b.tile([128, 4 * BL], f32, tag="cmb", name="pxh")
            combine(C_y, 0, 4, pxh, 0)
            nc.vector.tensor_copy(xhy_sb[:], pxh[:])

            drain_edots(2)

            # ---- 4) gates = x_h @ [Wi|Wf|Wo|Wg] (col-tiled), interleaved
            # with attention produce/e-dot; combine into gactT layout
            gT_ps = ps_g.tile([128, 4 * UC * BL], f32, tag="gT")
            for g in range(4):
                if g < 3:
                    produce(2 * g + 2)
                    produce(2 * g + 3)
                g_ps = ps_a.tile([128, U], f32, tag="mm", name="g_ps")
                for r in range(3):
                    for hh in range(2):
                        for g4 in range(4):
                            k = 4 * r + g4
                            lhsT = (xhy_sb[:, 8 * k : 8 * k + 8] if k < 4
                                    else rd[:, 8 * (k - 4) : 8 * (k - 4) + 8])
                            _mm(nc, g_ps[32 * g4 : 32 * g4 + 8, 512 * hh : 512 * (hh + 1)],
                                lhsT,
                                w4_sb[k][:, U * g + 512 * hh : U * g + 512 * (hh + 1)],
                                start=(r == 0), stop=(r == 2),
                                tile_position=(0, 32 * g4))
                C_g = c_pool.tile([128, U], bf16, tag="c", name="C_g")
                if g % 2 == 1:
                    nc.scalar.copy(C_g[:], g_ps[:])
                else:
                    nc.vector.tensor_copy(C_g[:], g_ps[:])
                combine(C_g, 0, UC, gT_ps, 64 * g)
                drain_edots(min(2 * g + 2, UC))
            # bias + activations on the transposed gates
            nc.vector.tensor_add(gsum[:], gT_ps[:], b4b_sb[:])
            nc.scalar.activation(gact[:, 0 : 3 * 64], gsum[:, 0 : 3 * 64],
                                 AF.Sigmoid)
            nc.scalar.activation(gact[:, 3 * 64 : 4 * 64],
                                 gsum[:, 3 * 64 : 4 * 64], AF.Tanh)

            drain_edots(UC)

            # ---- 5) e = sigmoid(e_dot + be2); softmax via exp-poly
            C_e = c_pool.tile([128, S], bf16, tag="c", name="C_e")
            nc.vector.tensor_copy(C_e[:], e_ps[:])
            e2_ps = ps_b.tile([BL, S], f32, tag="cmb", name="e2_ps")
            _mm(nc, e2_ps[:], E_sb[:], C_e[:], start=True, stop=True)
            nc.scalar.activation(esig[:], e2_ps[:], AF.Sigmoid,
                                 bias=be2r_sb[:, 0:1])
            c0, c1, c2, c3, c4 = [float(c) for c in _EXP_C]
            nc.vector.tensor_scalar(er[:], esig[:], c4, c3, ALU.mult, ALU.add)
            nc.vector.tensor_mul(eq[:], er[:], esig[:])
            nc.vector.tensor_scalar(er[:], eq[:], 1.0, c2, ALU.mult, ALU.add)
            nc.vector.tensor_mul(eq[:], er[:], esig[:])
            nc.vector.tensor_scalar(er[:], eq[:], 1.0, c1, ALU.mult, ALU.add)
            nc.vector.tensor_mul(eq[:], er[:], esig[:])
            nc.vector.tensor_scalar(er[:], eq[:], 1.0, c0, ALU.mult, ALU.add)
            nc.vector.tensor_reduce(den[:], er[:], mybir.AxisListType.X, ALU.add)
            nc.vector.reciprocal(rden[:], den[:])
            # fold 1/den into a before bf16 cast -> context needs no rescale
            nc.vector.tensor_scalar_mul(eq[:], er[:], rden[:])
            nc.vector.tensor_copy(ea_bf[:], eq[:])
            psA = ps_b.tile([128, 16], bf16, tag="cmb", name="psA")
            for sc in range(2):
                nc.tensor.transpose(
                    psA[:, 8 * sc : 8 * sc + 8], ea_bf[:, 128 * sc : 128 * (sc + 1)],
                    id8[:])
                nc.vector.tensor_copy(
                    A_ld[:, 8 * sc : 8 * sc + 17 * 7 + 1 : 17], psA[:, 8 * sc : 8 * sc + 8])

            # ---- 6) context c = (A^T @ h) (col-tiled over ci), combine
            cx_ps = ps_a.tile([128, U], f32, tag="mm", name="cx_ps")
            for p in range(4):
                for hh in range(2):
                    for g4 in range(4):
                        ci = 4 * g4 + p
                        _mm(nc, cx_ps[32 * g4 : 32 * g4 + 8, 512 * hh : 512 * (hh + 1)],
                            A_ld[:, 8 * ci : 8 * ci + 8],
                            h_tiles[ci][:, 512 * hh : 512 * (hh + 1)],
                            start=(p == 0), stop=(p == 3),
                            tile_position=(0, 32 * g4))
            C_c = c_pool.tile([128, U], bf16, tag="c", name="C_c")
            nc.vector.tensor_copy(C_c[:], cx_ps[:])
            cT_ps = ps_b.tile([128, UC * BL], f32, tag="cmb", name="cT_ps")
            combine(C_c, 0, UC, cT_ps, 0)

            # ---- 7) LSTM cell in transposed [feat, b] layout
            gi = gact[:, 0:64]
            gf = gact[:, 64:128]
            go = gact[:, 128:192]
            gg = gact[:, 192:256]
            nc.vector.tensor_mul(t1f[:], gf, cT_ps[:])
            nc.vector.tensor_mul(t2f[:], gi, gg)
            nc.vector.tensor_add(cnew[:], t1f[:], t2f[:])
            nc.scalar.activation(thb[:], cnew[:], AF.Tanh)
            nc.vector.tensor_mul(wr[:], go, thb[:])

        # zero the PSUM pool buffers once: matmuls only ever write the four
        # 8-row col-group slices; the combine matmuls read all 128 rows, and
        # garbage (NaN) in unwritten rows would poison 0*NaN products.
        for _ in range(2):
            tz = ps_a.tile([128, U], f32, tag="mm", name="pzero")
            nc.vector.memset(tz[:], 0.0)
        tze = ps_e.tile([128, S], f32, tag="e", name="pzeroe")
        nc.vector.memset(tze[:], 0.0)

        assert nsteps % unroll == 0
        if static_loop:
            for it in range(nsteps // unroll):
                for j in range(unroll):
                    step_body(it * unroll + j, j)
        else:
            with tc.For_i(0, nsteps // unroll,
                  hint_engines=(mybir.EngineType.PE, mybir.EngineType.DVE,
                                mybir.EngineType.Activation)) as iv:
                base = nc.snap(iv * unroll)
                for j in range(unroll):
                    step_body(base + j, j)

    nc.finalize()
    return nc


# ---------------------------------------------------------------------------
# numpy-side input prep + SPMD execution

_NC_CACHE = {}
TRACE = False
TMPDIR = None
LAST_RESULTS = None


def _prep_shared(Wy1, by1, Wy2, by2, We1, be1, We2, be2, Wf, bfb, Wi, bi, Wg, bg,
                 Wo, bo):
    bf = ml_dtypes.bfloat16
    f = np.float32
    sh = {}
    Wsy = np.concatenate([Wy1, We1[U:]], axis=1)            # [1024, 2048]
    sh["Wsy"] = np.ascontiguousarray(Wsy.reshape(UC, 128, 2 * U)).astype(bf)
    sh["Wy2b"] = np.ascontiguousarray(Wy2.reshape(UC, 128, T)).astype(bf)
    W4 = np.concatenate([Wi, Wf, Wo, Wg], axis=1)           # [1536, 4096]
    sh["W4"] = np.ascontiguousarray(W4.reshape(TC4, 128, G)).astype(bf)
    sh["We1h"] = np.ascontiguousarray(We1[:U].reshape(UC, 128, U)).astype(bf)
    sh["We2c"] = np.ascontiguousarray(We2.reshape(UC, 128).T).astype(bf)
    E = np.zeros((128, BL), np.float32)
    for jj in range(4):
        for b in range(BL):
            E[32 * jj + b, b] = 1.0
    sh["Emat"] = E.astype(bf)
    sh["ones1"] = np.ones((1, BL), bf)
    sh["by2row"] = by2.reshape(1, T).astype(bf)
    # feature-transposed bias broadcasts: col = uc*8 + b
    by1T = by1.reshape(UC, 128).T                            # [128, UC]
    be1T = be1.reshape(UC, 128).T
    sh["by1b"] = np.repeat(by1T, BL, axis=1).astype(f)       # [128, UC*BL]
    sh["be1b"] = np.repeat(be1T, BL, axis=1).astype(f)
    b4 = np.concatenate([bi, bfb, bo, bg])                   # [4096]
    b4T = b4.reshape(4 * UC, 128).T                          # [128, 4*UC]
    sh["b4b"] = np.repeat(b4T, BL, axis=1).astype(bf)        # [128, 4*UC*BL]
    sh["be2r"] = np.full((BL, 1), float(be2[0]), f)
    return sh


def kernel(h, s_0, Wy1, by1, Wy2, by2, We1, be1, We2, be2,
           Wf, bf, Wi, bi, Wg, bg, Wo, bo, nsteps=S, unroll=8):
    h = np.asarray(h, np.float32)
    s_0 = np.asarray(s_0, np.float32)
    key = (nsteps, unroll)
    if key not in _NC_CACHE:
        _NC_CACHE[key] = build(nsteps=nsteps, unroll=unroll)
    nc = _NC_CACHE[key]

    sh = _prep_shared(Wy1, by1, Wy2, by2, We1, be1, We2, be2,
                      np.asarray(Wf), np.asarray(bf), np.asarray(Wi),
                      np.asarray(bi), np.asarray(Wg), np.asarray(bg),
                      np.asarray(Wo), np.asarray(bo))
    bfd = ml_dtypes.bfloat16
    in_maps = []
    for i in range(NCORES):
        hc = h[i * BL : (i + 1) * BL]                       # [8, 256, 1024]
        m = dict(sh)
        m["h_bf"] = np.ascontiguousarray(
            hc.reshape(BL, 2, 128, U).reshape(2 * BL, 128, U)).astype(bfd)
        m["hT_bf"] = np.ascontiguousarray(
            hc.transpose(2, 0, 1).reshape(UC, 128, BS)).astype(bfd)
        m["s0b"] = s_0[i * BL : (i + 1) * BL].astype(bfd)
        in_maps.append(m)

    res = run_bass_kernel_spmd(nc, in_maps, core_ids=list(range(NCORES)),
                               trace=TRACE, tmpdir=TMPDIR)
    global LAST_RESULTS
    LAST_RESULTS = res
    outs = [r["ys"].reshape(BL, S, T)[:, :nsteps, :] for r in res.results]
    full = np.concatenate(outs, axis=0)
    return full.astype(np.float32)


if __name__ == "__main__":
    print("building...")
    build(nsteps=4, unroll=4)
    print("build ok")


# revision 32
# speedup vs baseline: 1.6999x; 1.6999x over previous
"""Trainium2 Bass kernel for nn_DecoderAttentionLSTM (v2).

Data-parallel over 8 NeuronCores on the batch axis (BL=8 batches/core).
Per core, the 256-step decode scan runs locally with all weights and the
precomputed h_proj = h @ We1[:U] SBUF-resident in bf16; h streams from
DRAM each step (context matmul only).

v2 structure (vs v1):
  - All thin-M (M=8) matmuls are 4x column-tiled: 4 concurrent rhs
    streams into distinct 32-column PE groups (outputs at PSUM partition
    bases 0/32/64/96), quartering weight-streaming time.
  - The 4 col-group partials are summed AND transposed in one PE
    "combine" matmul per 128-col chunk: out[feat,b] = C_chunk.T @ E,
    where E[32j+b, b] = 1. This yields feature-major [128, b] layouts
    for y1/sproj/gates/c, so activations get per-partition ACT bias and
    the LSTM cell runs on [128, 64] tiles (16x fewer DVE cycles than
    [8, 1024]), and the state needs no final transpose.
  - Attention z-add / sigmoid / e-dot pipelined with the gate matmuls.
  - softmax exp() via degree-4 polynomial (sigmoid output in (0,1)), so
    only the Sigmoid/Tanh ACT table set is ever loaded.
"""

import sys

sys.path.insert(0, "/opt/trn_rl_repo")

from contextlib import ExitStack  # noqa: E402

import ml_dtypes  # noqa: E402
import numpy as np  # noqa: E402

import concourse.bass as bass  # noqa: E402
import concourse.mybir as mybir  # noqa: E402
import concourse.tile as tile  # noqa: E402
from concourse import bacc  # noqa: E402
from concourse.bass import ds, ts  # noqa: E402
from concourse.bass_utils import run_bass_kernel_spmd  # noqa: E402
from concourse.masks import make_identity  # noqa: E402

B, S, U, T = 64, 256, 1024, 512
NCORES = 8
BL = B // NCORES          # 8 local batches
UC = U // 128             # 8 u-chunks
TC4 = (T + U) // 128      # 12 k-chunks for the gate matmuls
G = 4 * U                 # 4096 gate outputs (i|f|o|g)
BS = BL * S               # 2048

bf16 = mybir.dt.bfloat16
f32 = mybir.dt.float32
AF = mybir.ActivationFunctionType
ALU = mybir.AluOpType

# degree-4 polynomial for exp(x) on [0, 1] (abs err ~ 3e-6, values >= 1)
_x = np.linspace(0.0, 1.0, 2001)
_EXP_C = np.polyfit(_x, np.exp(_x), 4)[::-1]  # c0..c4


def _mm(nc, out, lhsT, rhs, start, stop, tile_position=None):
    nc.tensor.matmul(out, lhsT, rhs, start=start, stop=stop,
                     skip_group_check=True, tile_position=tile_position)


def build(nsteps=S, unroll=8, static_loop=False):
    """Build the Bass module (same program for all 8 cores)."""
    nc = bacc.Bacc("TRN2", target_bir_lowering=False, debug=False)

    # ---- DRAM I/O (per-core shapes; wrapper does layout/casts in numpy)
    d_hbf = nc.dram_tensor("h_bf", [2 * BL, 128, U], bf16, kind="ExternalInput")
    d_hT = nc.dram_tensor("hT_bf", [UC, 128, BS], bf16, kind="ExternalInput")
    d_we1h = nc.dram_tensor("We1h", [UC, 128, U], bf16, kind="ExternalInput")
    d_wsy = nc.dram_tensor("Wsy", [UC, 128, 2 * U], bf16, kind="ExternalInput")
    d_wy2 = nc.dram_tensor("Wy2b", [UC, 128, T], bf16, kind="ExternalInput")
    d_w4 = nc.dram_tensor("W4", [TC4, 128, G], bf16, kind="ExternalInput")
    d_we2 = nc.dram_tensor("We2c", [128, UC], bf16, kind="ExternalInput")
    d_E = nc.dram_tensor("Emat", [128, BL], bf16, kind="ExternalInput")
    d_ones1 = nc.dram_tensor("ones1", [1, BL], bf16, kind="ExternalInput")
    d_by2row = nc.dram_tensor("by2row", [1, T], bf16, kind="ExternalInput")
    d_by1b = nc.dram_tensor("by1b", [128, UC * BL], f32, kind="ExternalInput")
    d_be1b = nc.dram_tensor("be1b", [128, UC * BL], f32, kind="ExternalInput")
    d_b4b = nc.dram_tensor("b4b", [128, 4 * UC * BL], bf16, kind="ExternalInput")
    d_be2r = nc.dram_tensor("be2r", [BL, 1], f32, kind="ExternalInput")
    d_s0 = nc.dram_tensor("s0b", [BL, U], bf16, kind="ExternalInput")
    d_out = nc.dram_tensor("ys", [BL, S * T], f32, kind="ExternalOutput")

    with tile.TileContext(nc) as tc, ExitStack() as ctx:
        # ================= static SBUF (persists for the whole kernel)
        st = ctx.enter_context(tc.tile_pool(name="static", bufs=1))
        wsy_sb = [st.tile([128, 2 * U], bf16, tag=f"wsy{k}", name=f"wsy{k}") for k in range(UC)]
        wy2_sb = [st.tile([128, T], bf16, tag=f"wy2{k}", name=f"wy2{k}") for k in range(UC)]
        w4_sb = [st.tile([128, G], bf16, tag=f"w4{k}", name=f"w4{k}") for k in range(TC4)]
        hproj_sb = [st.tile([128, BS], bf16, tag=f"hp{k}", name=f"hp{k}") for k in range(UC)]
        we2d_sb = [st.tile([128, 8 * BL], bf16, tag=f"we2d{k}", name=f"we2d{k}") for k in range(UC)]
        E_sb = st.tile([128, BL], bf16, tag="Emat")
        ones1_sb = st.tile([1, BL], bf16, tag="ones1")
        by2row_sb = st.tile([1, T], bf16, tag="by2row")
        by1b_sb = st.tile([128, UC * BL], f32, tag="by1b")
        be1b_sb = st.tile([128, UC * BL], f32, tag="be1b")
        b4b_sb = st.tile([128, 4 * UC * BL], bf16, tag="b4b")
        be2r_sb = st.tile([BL, 1], f32, tag="be2r")
        id8 = st.tile([8, 8], bf16, tag="id8")
        A_ld = st.tile([128, 128], bf16, tag="A_ld")
        sT = [st.tile([128, UC * BL], bf16, tag=f"sT{p}", name=f"sT{p}") for p in range(2)]
        y1pre = st.tile([128, UC * BL], f32, tag="y1pre")
        y1t_sb = st.tile([128, UC * BL], bf16, tag="y1t")
        sprojT_sb = st.tile([128, UC * BL], f32, tag="sprojT")
        xhy_sb = st.tile([128, 4 * BL], bf16, tag="xhy")
        y_sb = st.tile([BL, T], f32, tag="y_sb")
        gact = st.tile([128, 4 * UC * BL], bf16, tag="gact")
        esig = st.tile([BL, S], f32, tag="esig")
        er = st.tile([BL, S], f32, tag="er")
        eq = st.tile([BL, S], f32, tag="eq")
        ea_bf = st.tile([BL, S], bf16, tag="ea_bf")
        den = st.tile([BL, 1], f32, tag="den")
        rden = st.tile([BL, 1], f32, tag="rden")
        gsum = st.tile([128, 4 * UC * BL], bf16, tag="gsum")
        t1f = st.tile([128, UC * BL], f32, tag="t1f")
        t2f = st.tile([128, UC * BL], f32, tag="t2f")
        cnew = st.tile([128, UC * BL], f32, tag="cnew")
        thb = st.tile([128, UC * BL], bf16, tag="thb")

        # ================= init: load weights, build masks
        make_identity(nc, id8[:])
        nc.vector.memset(A_ld[:], 0.0)
        for k in range(UC):
            nc.sync.dma_start(wsy_sb[k][:], d_wsy[k])
            nc.sync.dma_start(wy2_sb[k][:], d_wy2[k])
        for k in range(TC4):
            nc.sync.dma_start(w4_sb[k][:], d_w4[k])
        nc.sync.dma_start(E_sb[:], d_E[:])
        nc.sync.dma_start(ones1_sb[:], d_ones1[:])
        nc.sync.dma_start(by2row_sb[:], d_by2row[:])
        nc.sync.dma_start(by1b_sb[:], d_by1b[:])
        nc.sync.dma_start(be1b_sb[:], d_be1b[:])
        nc.sync.dma_start(b4b_sb[:], d_b4b[:])
        nc.sync.dma_start(be2r_sb[:], d_be2r[:])

        with tc.tile_pool(name="initp", bufs=2) as initp:
            we2_stage = initp.tile([128, UC], bf16, tag="we2stage")
            nc.sync.dma_start(we2_stage[:], d_we2[:])
            # We2 block-diagonal lhsT tiles: we2d[uc][:, 9*b] = We2 chunk uc
            for k in range(UC):
                nc.vector.memset(we2d_sb[k][:], 0.0)
                for b in range(BL):
                    nc.vector.tensor_copy(
                        we2d_sb[k][:, 9 * b : 9 * b + 1], we2_stage[:, k : k + 1]
                    )
            # initial state: s0 -> sT[0]
            s_bf = initp.tile([BL, U], bf16, tag="s_bf")
            nc.sync.dma_start(s_bf[:], d_s0[:])
            with tc.tile_pool(name="ps_init", bufs=1, space="PSUM") as ps_init:
                psT0 = ps_init.tile([128, UC * BL], bf16, tag="tr0")
                for q in range(UC):
                    nc.tensor.transpose(
                        psT0[:, 8 * q : 8 * q + 8],
                        s_bf[:, 128 * q : 128 * (q + 1)], id8[:]
                    )
                nc.vector.tensor_copy(sT[0][:], psT0[:])

        # ================= h_proj = (h @ We1[:U])^T into SBUF-resident tiles
        with tc.tile_pool(name="hp_w", bufs=3) as hp_w, \
             tc.tile_pool(name="hp_r", bufs=3) as hp_r, \
             tc.tile_pool(name="hp_ps", bufs=2, space="PSUM") as hp_ps:
            for m in range(UC):
                for n in range(BS // 512):
                    ps = hp_ps.tile([128, 512], f32, tag="hp_ps", name="hp_ps")
                    for k in range(UC):
                        wt = hp_w.tile([128, 128], bf16, tag="hp_w", name="hp_w")
                        nc.sync.dma_start(wt[:], d_we1h[k, :, 128 * m : 128 * (m + 1)])
                        rt = hp_r.tile([128, 512], bf16, tag="hp_r", name="hp_r")
                        nc.sync.dma_start(rt[:], d_hT[k, :, 512 * n : 512 * (n + 1)])
                        _mm(nc, ps[:], wt[:], rt[:],
                            start=(k == 0), stop=(k == UC - 1))
                    nc.vector.tensor_copy(
                        hproj_sb[m][:, 512 * n : 512 * (n + 1)], ps[:])

        # ================= working pools for the scan
        # PSUM budget (8 banks): ps_a 2x[128,1024]f32 = 4, ps_b 2x(<=1) = 2,
        # ps_e 1, ps_g 1.
        ps_a = ctx.enter_context(tc.tile_pool(name="ps_a", bufs=2, space="PSUM"))
        ps_b = ctx.enter_context(tc.tile_pool(name="ps_b", bufs=2, space="PSUM"))
        ps_e = ctx.enter_context(tc.tile_pool(name="ps_e", bufs=1, space="PSUM"))
        ps_g = ctx.enter_context(tc.tile_pool(name="ps_g", bufs=1, space="PSUM"))
        z_pool = ctx.enter_context(tc.tile_pool(name="z_pool", bufs=4))
        h_pool = ctx.enter_context(tc.tile_pool(name="h_pool", bufs=4))
        c_pool = ctx.enter_context(tc.tile_pool(name="c_pool", bufs=2))

        def col_mm(out_ps, j, lhsT, rhs, start, stop):
            """Matmul into column-group j (PSUM partitions 32j..32j+8)."""
            _mm(nc, out_ps[32 * j : 32 * j + 8, :], lhsT, rhs, start, stop,
                tile_position=(0, 32 * j))

        def combine(src_sb, c0, n, out_ps, o0):
            """out_ps[:, o0+8c] = src_sb[:, 128c].T @ E  (sum 4 groups +
            transpose) for n consecutive 128-col chunks."""
            for c in range(n):
                _mm(nc, out_ps[:, o0 + 8 * c : o0 + 8 * c + 8],
                    src_sb[:, 128 * (c0 + c) : 128 * (c0 + c + 1)], E_sb[:],
                    start=True, stop=True)

        def step_body(step_ap, j):
            """One decode step. step_ap: dynamic step index (ScalarValue)."""
            rd = sT[j % 2]
            wr = sT[(j + 1) % 2]

            e1_tiles = []
            e_ps = ps_e.tile([128, S], f32, tag="e")
            edone = [0]

            def produce(uc):
                """z = hproj[uc] + sproj (DVE + 2 blocks on GPSIMD),
                e1 = sigmoid(z) (ACT)."""
                z_t = z_pool.tile([128, BS], bf16, tag="z", name="z_t")
                for b in range(BL):
                    eng = nc.vector
                    eng.tensor_scalar_add(
                        z_t[:, 256 * b : 256 * (b + 1)],
                        hproj_sb[uc][:, 256 * b : 256 * (b + 1)],
                        sprojT_sb[:, 8 * uc + b : 8 * uc + b + 1])
                e1_t = z_pool.tile([128, BS], bf16, tag="z", name="e1_t")
                nc.scalar.activation(e1_t[:], z_t[:], AF.Sigmoid)
                e1_tiles.append(e1_t)

            def edot_mm(uc, b):
                g = uc % 4
                _mm(nc, e_ps[32 * g : 32 * g + 8, :],
                    we2d_sb[uc][:, 8 * b : 8 * b + 8],
                    e1_tiles[uc][:, 256 * b : 256 * (b + 1)],
                    start=(uc < 4 and b == 0),
                    stop=(uc >= 4 and b == BL - 1),
                    tile_position=(0, 32 * g))

            def drain_edots(upto):
                """e[b,:] += We2[uc] . e1[uc][:, b-block]; column-group uc%4.
                Interleave batches across uc pairs so adjacent matmuls hit
                different column groups."""
                while edone[0] < upto:
                    e0 = edone[0]
                    if upto - e0 >= 2:
                        for b in range(BL):
                            edot_mm(e0, b)
                            edot_mm(e0 + 1, b)
                        edone[0] += 2
                    else:
                        for b in range(BL):
                            edot_mm(e0, b)
                        edone[0] += 1

            # ---- h prefetch for the context matmul (consumed at step end).
            # Issue DMAs in the ctx consumption order (group-interleaved).
            h_tiles = {}
            for p in range(4):
                for g4 in range(4):
                    ci = 4 * g4 + p
                    h_t = h_pool.tile([128, U], bf16, tag="h", name="h_t")
                    nc.gpsimd.dma_start(h_t[:], d_hbf[ci])
                    h_tiles[ci] = h_t

            # ---- 1) sproj = s @ We1_s (col-tiled), combine+transpose
            sp_ps = ps_a.tile([128, U], f32, tag="mm", name="sp_ps")
            for r in range(2):
                for hh in range(2):
                    for g4 in range(4):
                        k = 4 * r + g4
                        _mm(nc, sp_ps[32 * g4 : 32 * g4 + 8, 512 * hh : 512 * (hh + 1)],
                            rd[:, 8 * k : 8 * k + 8],
                            wsy_sb[k][:, U + 512 * hh : U + 512 * (hh + 1)],
                            start=(r == 0), stop=(r == 1),
                            tile_position=(0, 32 * g4))
            C1 = c_pool.tile([128, U], bf16, tag="c", name="C1")
            nc.vector.tensor_copy(C1[:], sp_ps[:])
            pT1 = ps_b.tile([128, UC * BL], f32, tag="cmb", name="pT1")
            combine(C1, 0, UC, pT1, 0)
            nc.vector.tensor_add(sprojT_sb[:], pT1[:], be1b_sb[:])

            # ---- start attention produce early (ucs 0..1)
            produce(0)
            produce(1)

            # ---- 2) y1 = s @ Wy1 (col-tiled), combine, tanh
            y1_ps = ps_a.tile([128, U], f32, tag="mm", name="y1_ps")
            for r in range(2):
                for hh in range(2):
                    for g4 in range(4):
                        k = 4 * r + g4
                        _mm(nc, y1_ps[32 * g4 : 32 * g4 + 8, 512 * hh : 512 * (hh + 1)],
                            rd[:, 8 * k : 8 * k + 8],
                            wsy_sb[k][:, 512 * hh : 512 * (hh + 1)],
                            start=(r == 0), stop=(r == 1),
                            tile_position=(0, 32 * g4))
            C0 = c_pool.tile([128, U], bf16, tag="c", name="C0")
            nc.vector.tensor_copy(C0[:], y1_ps[:])
            pT0 = ps_b.tile([128, UC * BL], f32, tag="cmb", name="pT0")
            combine(C0, 0, UC, pT0, 0)
            nc.vector.tensor_add(y1pre[:], pT0[:], by1b_sb[:])
            nc.scalar.activation(y1t_sb[:], y1pre[:], AF.Tanh)

            # ---- 3) y = y1 @ Wy2 + by2 (col-tiled + bias-in-stream)
            y_ps = ps_a.tile([128, T], f32, tag="mm", name="y_ps")
            for r in range(2):
                for g4 in range(4):
                    k = 4 * r + g4
                    col_mm(y_ps, g4, y1t_sb[:, 8 * k : 8 * k + 8],
                           wy2_sb[k][:], start=(r == 0),
                           stop=(r == 1 and g4 > 0))
            # bias row into group 0 (counted once across the group sum)
            _mm(nc, y_ps[0:8, :], ones1_sb[:], by2row_sb[:],
                start=False, stop=True)
            C_y = c_pool.tile([128, T], bf16, tag="c", name="C_y")
            nc.vector.tensor_copy(C_y[:], y_ps[:])
            # row-major y for the output DMA
            y2_ps = ps_b.tile([BL, T], f32, tag="cmb", name="y2_ps")
            _mm(nc, y2_ps[:], E_sb[:], C_y[:], start=True, stop=True)
            nc.scalar.copy(y_sb[:], y2_ps[:])
            nc.sync.dma_start(d_out[:, ts(step_ap, T)], y_sb[:])
            # transposed y chunks -> xhy (gate lhsT)
            pxh = ps_b.tile([128, 4 * BL], f32, tag="cmb", name="pxh")
            combine(C_y, 0, 4, pxh, 0)
            nc.vector.tensor_copy(xhy_sb[:], pxh[:])

            drain_edots(2)

            # ---- 4) gates = x_h @ [Wi|Wf|Wo|Wg] (col-tiled), interleaved
            # with attention produce/e-dot; combine into gactT layout
            gT_ps = ps_g.tile([128, 4 * UC * BL], f32, tag="gT")
            for g in range(4):
                if g < 3:
                    produce(2 * g + 2)
                    produce(2 * g + 3)
                g_ps = ps_a.tile([128, U], f32, tag="mm", name="g_ps")
                for r in range(3):
                    for hh in range(2):
                        for g4 in range(4):
                            k = 4 * r + g4
                            lhsT = (xhy_sb[:, 8 * k : 8 * k + 8] if k < 4
                                    else rd[:, 8 * (k - 4) : 8 * (k - 4) + 8])
                            _mm(nc, g_ps[32 * g4 : 32 * g4 + 8, 512 * hh : 512 * (hh + 1)],
                                lhsT,
                                w4_sb[k][:, U * g + 512 * hh : U * g + 512 * (hh + 1)],
                                start=(r == 0), stop=(r == 2),
                                tile_position=(0, 32 * g4))
                C_g = c_pool.tile([128, U], bf16, tag="c", name="C_g")
                if g % 2 == 1:
                    nc.scalar.copy(C_g[:], g_ps[:])
                else:
                    nc.vector.tensor_copy(C_g[:], g_ps[:])
                combine(C_g, 0, UC, gT_ps, 64 * g)
                drain_edots(min(2 * g + 2, UC))
            # bias + activations on the transposed gates
            nc.vector.tensor_add(gsum[:], gT_ps[:], b4b_sb[:])
            nc.scalar.activation(gact[:, 0 : 3 * 64], gsum[:, 0 : 3 * 64],
                                 AF.Sigmoid)
            nc.scalar.activation(gact[:, 3 * 64 : 4 * 64],
                                 gsum[:, 3 * 64 : 4 * 64], AF.Tanh)

            drain_edots(UC)

            # ---- 5) e = sigmoid(e_dot + be2); softmax via exp-poly
            C_e = c_pool.tile([128, S], bf16, tag="c", name="C_e")
            nc.vector.tensor_copy(C_e[:], e_ps[:])
            e2_ps = ps_b.tile([BL, S], f32, tag="cmb", name="e2_ps")
            _mm(nc, e2_ps[:], E_sb[:], C_e[:], start=True, stop=True)
            nc.scalar.activation(esig[:], e2_ps[:], AF.Sigmoid,
                                 bias=be2r_sb[:, 0:1])
            c0, c1, c2, c3, c4 = [float(c) for c in _EXP_C]
            nc.vector.tensor_scalar(er[:], esig[:], c4, c3, ALU.mult, ALU.add)
            nc.vector.tensor_mul(eq[:], er[:], esig[:])
            nc.vector.tensor_scalar(er[:], eq[:], 1.0, c2, ALU.mult, ALU.add)
            nc.vector.tensor_mul(eq[:], er[:], esig[:])
            nc.vector.tensor_scalar(er[:], eq[:], 1.0, c1, ALU.mult, ALU.add)
            nc.vector.tensor_mul(eq[:], er[:], esig[:])
            nc.vector.tensor_scalar(er[:], eq[:], 1.0, c0, ALU.mult, ALU.add)
            nc.vector.tensor_reduce(den[:], er[:], mybir.AxisListType.X, ALU.add)
            nc.vector.reciprocal(rden[:], den[:])
            # fold 1/den into a before bf16 cast -> context needs no rescale
            nc.vector.tensor_scalar_mul(eq[:], er[:], rden[:])
            nc.vector.tensor_copy(ea_bf[:], eq[:])
            psA = ps_b.tile([128, 16], bf16, tag="cmb", name="psA")
            for sc in range(2):
                nc.tensor.transpose(
                    psA[:, 8 * sc : 8 * sc + 8], ea_bf[:, 128 * sc : 128 * (sc + 1)],
                    id8[:])
                nc.vector.tensor_copy(
                    A_ld[:, 8 * sc : 8 * sc + 17 * 7 + 1 : 17], psA[:, 8 * sc : 8 * sc + 8])

            # ---- 6) context c = (A^T @ h) (col-tiled over ci), combine
            cx_ps = ps_a.tile([128, U], f32, tag="mm", name="cx_ps")
            for p in range(4):
                for hh in range(2):
                    for g4 in range(4):
                        ci = 4 * g4 + p
                        _mm(nc, cx_ps[32 * g4 : 32 * g4 + 8, 512 * hh : 512 * (hh + 1)],
                            A_ld[:, 8 * ci : 8 * ci + 8],
                            h_tiles[ci][:, 512 * hh : 512 * (hh + 1)],
                            start=(p == 0), stop=(p == 3),
                            tile_position=(0, 32 * g4))
            C_c = c_pool.tile([128, U], bf16, tag="c", name="C_c")
            nc.vector.tensor_copy(C_c[:], cx_ps[:])
            cT_ps = ps_b.tile([128, UC * BL], f32, tag="cmb", name="cT_ps")
            combine(C_c, 0, UC, cT_ps, 0)

            # ---- 7) LSTM cell in transposed [feat, b] layout
            gi = gact[:, 0:64]
            gf = gact[:, 64:128]
            go = gact[:, 128:192]
            gg = gact[:, 192:256]
            nc.vector.tensor_mul(t1f[:], gf, cT_ps[:])
            nc.vector.tensor_mul(t2f[:], gi, gg)
            nc.vector.tensor_add(cnew[:], t1f[:], t2f[:])
            nc.scalar.activation(thb[:], cnew[:], AF.Tanh)
            nc.vector.tensor_mul(wr[:], go, thb[:])

        # zero the PSUM pool buffers once: matmuls only ever write the four
        # 8-row col-group slices; the combine matmuls read all 128 rows, and
        # garbage (NaN) in unwritten rows would poison 0*NaN products.
        for _ in range(2):
            tz = ps_a.tile([128, U], f32, tag="mm", name="pzero")
            nc.vector.memset(tz[:], 0.0)
        tze = ps_e.tile([128, S], f32, tag="e", name="pzeroe")
        nc.vector.memset(tze[:], 0.0)

        assert nsteps % unroll == 0
        if static_loop:
            for it in range(nsteps // unroll):
                for j in range(unroll):
                    step_body(it * unroll + j, j)
        else:
            with tc.For_i(0, nsteps // unroll,
                  hint_engines=(mybir.EngineType.PE, mybir.EngineType.DVE,
                                mybir.EngineType.Activation)) as iv:
                base = nc.snap(iv * unroll)
                for j in range(unroll):
                    step_body(base + j, j)

    nc.finalize()
    return nc


# ---------------------------------------------------------------------------
# numpy-side input prep + SPMD execution

_NC_CACHE = {}
TRACE = False
TMPDIR = None
LAST_RESULTS = None


def _prep_shared(Wy1, by1, Wy2, by2, We1, be1, We2, be2, Wf, bfb, Wi, bi, Wg, bg,
                 Wo, bo):
    bf = ml_dtypes.bfloat16
    f = np.float32
    sh = {}
    Wsy = np.concatenate([Wy1, We1[U:]], axis=1)            # [1024, 2048]
    sh["Wsy"] = np.ascontiguousarray(Wsy.reshape(UC, 128, 2 * U)).astype(bf)
    sh["Wy2b"] = np.ascontiguousarray(Wy2.reshape(UC, 128, T)).astype(bf)
    W4 = np.concatenate([Wi, Wf, Wo, Wg], axis=1)           # [1536, 4096]
    sh["W4"] = np.ascontiguousarray(W4.reshape(TC4, 128, G)).astype(bf)
    sh["We1h"] = np.ascontiguousarray(We1[:U].reshape(UC, 128, U)).astype(bf)
    sh["We2c"] = np.ascontiguousarray(We2.reshape(UC, 128).T).astype(bf)
    E = np.zeros((128, BL), np.float32)
    for jj in range(4):
        for b in range(BL):
            E[32 * jj + b, b] = 1.0
    sh["Emat"] = E.astype(bf)
    sh["ones1"] = np.ones((1, BL), bf)
    sh["by2row"] = by2.reshape(1, T).astype(bf)
    # feature-transposed bias broadcasts: col = uc*8 + b
    by1T = by1.reshape(UC, 128).T                            # [128, UC]
    be1T = be1.reshape(UC, 128).T
    sh["by1b"] = np.repeat(by1T, BL, axis=1).astype(f)       # [128, UC*BL]
    sh["be1b"] = np.repeat(be1T, BL, axis=1).astype(f)
    b4 = np.concatenate([bi, bfb, bo, bg])                   # [4096]
    b4T = b4.reshape(4 * UC, 128).T                          # [128, 4*UC]
    sh["b4b"] = np.repeat(b4T, BL, axis=1).astype(bf)        # [128, 4*UC*BL]
    sh["be2r"] = np.full((BL, 1), float(be2[0]), f)
    return sh


def kernel(h, s_0, Wy1, by1, Wy2, by2, We1, be1, We2, be2,
           Wf, bf, Wi, bi, Wg, bg, Wo, bo, nsteps=S, unroll=8):
    h = np.asarray(h, np.float32)
    s_0 = np.asarray(s_0, np.float32)
    key = (nsteps, unroll)
    if key not in _NC_CACHE:
        _NC_CACHE[key] = build(nsteps=nsteps, unroll=unroll)
    nc = _NC_CACHE[key]

    sh = _prep_shared(Wy1, by1, Wy2, by2, We1, be1, We2, be2,
                      np.asarray(Wf), np.asarray(bf), np.asarray(Wi),
                      np.asarray(bi), np.asarray(Wg), np.asarray(bg),
                      np.asarray(Wo), np.asarray(bo))
    bfd = ml_dtypes.bfloat16
    in_maps = []
    for i in range(NCORES):
        hc = h[i * BL : (i + 1) * BL]                       # [8, 256, 1024]
        m = dict(sh)
        m["h_bf"] = np.ascontiguousarray(
            hc.reshape(BL, 2, 128, U).reshape(2 * BL, 128, U)).astype(bfd)
        m["hT_bf"] = np.ascontiguousarray(
            hc.transpose(2, 0, 1).reshape(UC, 128, BS)).astype(bfd)
        m["s0b"] = s_0[i * BL : (i + 1) * BL].astype(bfd)
        in_maps.append(m)

    res = run_bass_kernel_spmd(nc, in_maps, core_ids=list(range(NCORES)),
                               trace=TRACE, tmpdir=TMPDIR)
    global LAST_RESULTS
    LAST_RESULTS = res
    outs = [r["ys"].reshape(BL, S, T)[:, :nsteps, :] for r in res.results]
    full = np.concatenate(outs, axis=0)
    return full.astype(np.float32)


if __name__ == "__main__":
    print("building...")
    build(nsteps=4, unroll=4)
    print("build ok")


# revision 34
# speedup vs baseline: 1.8096x; 1.0645x over previous
"""Trainium2 Bass kernel for nn_DecoderAttentionLSTM (v2).

Data-parallel over 8 NeuronCores on the batch axis (BL=8 batches/core).
Per core, the 256-step decode scan runs locally with all weights and the
precomputed h_proj = h @ We1[:U] SBUF-resident in bf16; h streams from
DRAM each step (context matmul only).

v2 structure (vs v1):
  - All thin-M (M=8) matmuls are 4x column-tiled: 4 concurrent rhs
    streams into distinct 32-column PE groups (outputs at PSUM partition
    bases 0/32/64/96), quartering weight-streaming time.
  - The 4 col-group partials are summed AND transposed in one PE
    "combine" matmul per 128-col chunk: out[feat,b] = C_chunk.T @ E,
    where E[32j+b, b] = 1. This yields feature-major [128, b] layouts
    for y1/sproj/gates/c, so activations get per-partition ACT bias and
    the LSTM cell runs on [128, 64] tiles (16x fewer DVE cycles than
    [8, 1024]), and the state needs no final transpose.
  - Attention z-add / sigmoid / e-dot pipelined with the gate matmuls.
  - softmax exp() via degree-4 polynomial (sigmoid output in (0,1)), so
    only the Sigmoid/Tanh ACT table set is ever loaded.
"""

import sys

sys.path.insert(0, "/opt/trn_rl_repo")

from contextlib import ExitStack  # noqa: E402

import ml_dtypes  # noqa: E402
import numpy as np  # noqa: E402

import concourse.bass as bass  # noqa: E402
import concourse.mybir as mybir  # noqa: E402
import concourse.tile as tile  # noqa: E402
from concourse import bacc  # noqa: E402
from concourse.bass import ds, ts  # noqa: E402
from concourse.bass_utils import run_bass_kernel_spmd  # noqa: E402
from concourse.masks import make_identity  # noqa: E402

B, S, U, T = 64, 256, 1024, 512
NCORES = 8
BL = B // NCORES          # 8 local batches
UC = U // 128             # 8 u-chunks
TC4 = (T + U) // 128      # 12 k-chunks for the gate matmuls
G = 4 * U                 # 4096 gate outputs (i|f|o|g)
BS = BL * S               # 2048

bf16 = mybir.dt.bfloat16
f32 = mybir.dt.float32
AF = mybir.ActivationFunctionType
ALU = mybir.AluOpType

# degree-4 polynomial for exp(x) on [0, 1] (abs err ~ 3e-6, values >= 1)
_x = np.linspace(0.0, 1.0, 2001)
_EXP_C = np.polyfit(_x, np.exp(_x), 4)[::-1]  # c0..c4


def _mm(nc, out, lhsT, rhs, start, stop, tile_position=None):
    nc.tensor.matmul(out, lhsT, rhs, start=start, stop=stop,
                     skip_group_check=True, tile_position=tile_position)


def build(nsteps=S, unroll=8, static_loop=False):
    """Build the Bass module (same program for all 8 cores)."""
    nc = bacc.Bacc("TRN2", target_bir_lowering=False, debug=False)

    # ---- DRAM I/O (per-core shapes; wrapper does layout/casts in numpy)
    d_hbf = nc.dram_tensor("h_bf", [2 * BL, 128, U], bf16, kind="ExternalInput")
    d_hT = nc.dram_tensor("hT_bf", [UC, 128, BS], bf16, kind="ExternalInput")
    d_we1h = nc.dram_tensor("We1h", [UC, 128, U], bf16, kind="ExternalInput")
    d_wsy = nc.dram_tensor("Wsy", [UC, 128, 2 * U], bf16, kind="ExternalInput")
    d_wy2 = nc.dram_tensor("Wy2b", [UC, 128, T], bf16, kind="ExternalInput")
    d_w4 = nc.dram_tensor("W4", [TC4, 128, G], bf16, kind="ExternalInput")
    d_we2 = nc.dram_tensor("We2c", [128, UC], bf16, kind="ExternalInput")
    d_E = nc.dram_tensor("Emat", [128, BL], bf16, kind="ExternalInput")
    d_ones1 = nc.dram_tensor("ones1", [1, BL], bf16, kind="ExternalInput")
    d_by2row = nc.dram_tensor("by2row", [1, T], bf16, kind="ExternalInput")
    d_by1b = nc.dram_tensor("by1b", [128, UC * BL], f32, kind="ExternalInput")
    d_be1b = nc.dram_tensor("be1b", [128, UC * BL], f32, kind="ExternalInput")
    d_b4b = nc.dram_tensor("b4b", [128, 4 * UC * BL], bf16, kind="ExternalInput")
    d_be2r = nc.dram_tensor("be2r", [BL, 1], f32, kind="ExternalInput")
    d_s0 = nc.dram_tensor("s0b", [BL, U], bf16, kind="ExternalInput")
    d_out = nc.dram_tensor("ys", [BL, S * T], f32, kind="ExternalOutput")

    with tile.TileContext(nc) as tc, ExitStack() as ctx:
        # ================= static SBUF (persists for the whole kernel)
        st = ctx.enter_context(tc.tile_pool(name="static", bufs=1))
        wsy_sb = [st.tile([128, 2 * U], bf16, tag=f"wsy{k}", name=f"wsy{k}") for k in range(UC)]
        wy2_sb = [st.tile([128, T], bf16, tag=f"wy2{k}", name=f"wy2{k}") for k in range(UC)]
        w4_sb = [st.tile([128, G], bf16, tag=f"w4{k}", name=f"w4{k}") for k in range(TC4)]
        hproj_sb = [st.tile([128, BS], bf16, tag=f"hp{k}", name=f"hp{k}") for k in range(UC)]
        we2d_sb = [st.tile([128, 8 * BL], bf16, tag=f"we2d{k}", name=f"we2d{k}") for k in range(UC)]
        E_sb = st.tile([128, BL], bf16, tag="Emat")
        ones1_sb = st.tile([1, BL], bf16, tag="ones1")
        by2row_sb = st.tile([1, T], bf16, tag="by2row")
        by1b_sb = st.tile([128, UC * BL], f32, tag="by1b")
        be1b_sb = st.tile([128, UC * BL], f32, tag="be1b")
        b4b_sb = st.tile([128, 4 * UC * BL], bf16, tag="b4b")
        be2r_sb = st.tile([BL, 1], f32, tag="be2r")
        id8 = st.tile([8, 8], bf16, tag="id8")
        A_ld = st.tile([128, 128], bf16, tag="A_ld")
        sT = [st.tile([128, UC * BL], bf16, tag=f"sT{p}", name=f"sT{p}") for p in range(2)]
        y1pre = st.tile([128, UC * BL], f32, tag="y1pre")
        y1t_sb = st.tile([128, UC * BL], bf16, tag="y1t")
        sprojT_sb = st.tile([128, UC * BL], f32, tag="sprojT")
        xhy_sb = st.tile([128, 4 * BL], bf16, tag="xhy")
        y_sb = st.tile([BL, T], f32, tag="y_sb")
        gact = st.tile([128, 4 * UC * BL], bf16, tag="gact")
        esig = st.tile([BL, S], f32, tag="esig")
        er = st.tile([BL, S], f32, tag="er")
        eq = st.tile([BL, S], f32, tag="eq")
        ea_bf = st.tile([BL, S], bf16, tag="ea_bf")
        den = st.tile([BL, 1], f32, tag="den")
        rden = st.tile([BL, 1], f32, tag="rden")
        gsum = st.tile([128, 4 * UC * BL], bf16, tag="gsum")
        t1f = st.tile([128, UC * BL], f32, tag="t1f")
        t2f = st.tile([128, UC * BL], f32, tag="t2f")
        cnew = st.tile([128, UC * BL], f32, tag="cnew")
        thb = st.tile([128, UC * BL], bf16, tag="thb")

        # ================= init: load weights, build masks
        make_identity(nc, id8[:])
        nc.vector.memset(A_ld[:], 0.0)
        for k in range(UC):
            nc.sync.dma_start(wsy_sb[k][:], d_wsy[k])
            nc.sync.dma_start(wy2_sb[k][:], d_wy2[k])
        for k in range(TC4):
            nc.sync.dma_start(w4_sb[k][:], d_w4[k])
        nc.sync.dma_start(E_sb[:], d_E[:])
        nc.sync.dma_start(ones1_sb[:], d_ones1[:])
        nc.sync.dma_start(by2row_sb[:], d_by2row[:])
        nc.sync.dma_start(by1b_sb[:], d_by1b[:])
        nc.sync.dma_start(be1b_sb[:], d_be1b[:])
        nc.sync.dma_start(b4b_sb[:], d_b4b[:])
        nc.sync.dma_start(be2r_sb[:], d_be2r[:])

        with tc.tile_pool(name="initp", bufs=2) as initp:
            we2_stage = initp.tile([128, UC], bf16, tag="we2stage")
            nc.sync.dma_start(we2_stage[:], d_we2[:])
            # We2 block-diagonal lhsT tiles: we2d[uc][:, 9*b] = We2 chunk uc
            for k in range(UC):
                nc.vector.memset(we2d_sb[k][:], 0.0)
                for b in range(BL):
                    nc.vector.tensor_copy(
                        we2d_sb[k][:, 9 * b : 9 * b + 1], we2_stage[:, k : k + 1]
                    )
            # initial state: s0 -> sT[0]
            s_bf = initp.tile([BL, U], bf16, tag="s_bf")
            nc.sync.dma_start(s_bf[:], d_s0[:])
            with tc.tile_pool(name="ps_init", bufs=1, space="PSUM") as ps_init:
                psT0 = ps_init.tile([128, UC * BL], bf16, tag="tr0")
                for q in range(UC):
                    nc.tensor.transpose(
                        psT0[:, 8 * q : 8 * q + 8],
                        s_bf[:, 128 * q : 128 * (q + 1)], id8[:]
                    )
                nc.vector.tensor_copy(sT[0][:], psT0[:])

        # ================= h_proj = (h @ We1[:U])^T into SBUF-resident tiles
        with tc.tile_pool(name="hp_w", bufs=3) as hp_w, \
             tc.tile_pool(name="hp_r", bufs=3) as hp_r, \
             tc.tile_pool(name="hp_ps", bufs=2, space="PSUM") as hp_ps:
            for m in range(UC):
                for n in range(BS // 512):
                    ps = hp_ps.tile([128, 512], f32, tag="hp_ps", name="hp_ps")
                    for k in range(UC):
                        wt = hp_w.tile([128, 128], bf16, tag="hp_w", name="hp_w")
                        nc.sync.dma_start(wt[:], d_we1h[k, :, 128 * m : 128 * (m + 1)])
                        rt = hp_r.tile([128, 512], bf16, tag="hp_r", name="hp_r")
                        nc.sync.dma_start(rt[:], d_hT[k, :, 512 * n : 512 * (n + 1)])
                        _mm(nc, ps[:], wt[:], rt[:],
                            start=(k == 0), stop=(k == UC - 1))
                    nc.vector.tensor_copy(
                        hproj_sb[m][:, 512 * n : 512 * (n + 1)], ps[:])

        # ================= working pools for the scan
        # PSUM budget (8 banks): ps_a 2x[128,1024]f32 = 4, ps_b 2x(<=1) = 2,
        # ps_e 1, ps_g 1.
        ps_a = ctx.enter_context(tc.tile_pool(name="ps_a", bufs=2, space="PSUM"))
        ps_b = ctx.enter_context(tc.tile_pool(name="ps_b", bufs=2, space="PSUM"))
        ps_e = ctx.enter_context(tc.tile_pool(name="ps_e", bufs=1, space="PSUM"))
        ps_g = ctx.enter_context(tc.tile_pool(name="ps_g", bufs=1, space="PSUM"))
        z_pool = ctx.enter_context(tc.tile_pool(name="z_pool", bufs=4))
        h_pool = ctx.enter_context(tc.tile_pool(name="h_pool", bufs=4))
        c_pool = ctx.enter_context(tc.tile_pool(name="c_pool", bufs=2))

        def col_mm(out_ps, j, lhsT, rhs, start, stop):
            """Matmul into column-group j (PSUM partitions 32j..32j+8)."""
            _mm(nc, out_ps[32 * j : 32 * j + 8, :], lhsT, rhs, start, stop,
                tile_position=(0, 32 * j))

        def combine(src_sb, c0, n, out_ps, o0):
            """out_ps[:, o0+8c] = src_sb[:, 128c].T @ E  (sum 4 groups +
            transpose) for n consecutive 128-col chunks."""
            for c in range(n):
                _mm(nc, out_ps[:, o0 + 8 * c : o0 + 8 * c + 8],
                    src_sb[:, 128 * (c0 + c) : 128 * (c0 + c + 1)], E_sb[:],
                    start=True, stop=True)

        def step_body(step_ap, j):
            """One decode step. step_ap: dynamic step index (ScalarValue)."""
            rd = sT[j % 2]
            wr = sT[(j + 1) % 2]

            e1_tiles = []
            e_ps = ps_e.tile([128, S], f32, tag="e")
            edone = [0]

            def produce(uc):
                """z = hproj[uc] + sproj (DVE + 2 blocks on GPSIMD),
                e1 = sigmoid(z) (ACT)."""
                z_t = z_pool.tile([128, BS], bf16, tag="z", name="z_t")
                for b in range(BL):
                    eng = nc.vector
                    eng.tensor_scalar_add(
                        z_t[:, 256 * b : 256 * (b + 1)],
                        hproj_sb[uc][:, 256 * b : 256 * (b + 1)],
                        sprojT_sb[:, 8 * uc + b : 8 * uc + b + 1])
                e1_t = z_pool.tile([128, BS], bf16, tag="z", name="e1_t")
                nc.scalar.activation(e1_t[:], z_t[:], AF.Sigmoid)
                e1_tiles.append(e1_t)

            def edot_mm(uc, b):
                g = uc % 4
                _mm(nc, e_ps[32 * g : 32 * g + 8, :],
                    we2d_sb[uc][:, 8 * b : 8 * b + 8],
                    e1_tiles[uc][:, 256 * b : 256 * (b + 1)],
                    start=(uc < 4 and b == 0),
                    stop=(uc >= 4 and b == BL - 1),
                    tile_position=(0, 32 * g))

            def drain_edots(upto):
                """e[b,:] += We2[uc] . e1[uc][:, b-block]; column-group uc%4.
                Interleave batches across uc pairs so adjacent matmuls hit
                different column groups."""
                while edone[0] < upto:
                    for b in range(BL):
                        edot_mm(edone[0], b)
                    edone[0] += 1

            # ---- h prefetch for the context matmul (consumed at step end).
            # Issue DMAs in the ctx consumption order (group-interleaved).
            h_tiles = {}
            for p in range(4):
                for g4 in range(4):
                    ci = 4 * g4 + p
                    h_t = h_pool.tile([128, U], bf16, tag="h", name="h_t")
                    nc.gpsimd.dma_start(h_t[:], d_hbf[ci])
                    h_tiles[ci] = h_t

            # ---- 1) sproj = s @ We1_s (col-tiled), combine+transpose
            sp_ps = ps_a.tile([128, U], f32, tag="mm", name="sp_ps")
            for r in range(2):
                for hh in range(2):
                    for g4 in range(4):
                        k = 4 * r + g4
                        _mm(nc, sp_ps[32 * g4 : 32 * g4 + 8, 512 * hh : 512 * (hh + 1)],
                            rd[:, 8 * k : 8 * k + 8],
                            wsy_sb[k][:, U + 512 * hh : U + 512 * (hh + 1)],
                            start=(r == 0), stop=(r == 1),
                            tile_position=(0, 32 * g4))
            C1 = c_pool.tile([128, U], bf16, tag="c", name="C1")
            nc.vector.tensor_copy(C1[:], sp_ps[:])
            pT1 = ps_b.tile([128, UC * BL], f32, tag="cmb", name="pT1")
            combine(C1, 0, UC, pT1, 0)
            nc.vector.tensor_add(sprojT_sb[:], pT1[:], be1b_sb[:])

            # ---- start attention produce early (ucs 0..1)
            produce(0)
            produce(1)

            # ---- 2) y1 = s @ Wy1 (col-tiled), combine, tanh
            y1_ps = ps_a.tile([128, U], f32, tag="mm", name="y1_ps")
            for r in range(2):
                for hh in range(2):
                    for g4 in range(4):
                        k = 4 * r + g4
                        _mm(nc, y1_ps[32 * g4 : 32 * g4 + 8, 512 * hh : 512 * (hh + 1)],
                            rd[:, 8 * k : 8 * k + 8],
                            wsy_sb[k][:, 512 * hh : 512 * (hh + 1)],
                            start=(r == 0), stop=(r == 1),
                            tile_position=(0, 32 * g4))
            C0 = c_pool.tile([128, U], bf16, tag="c", name="C0")
            nc.vector.tensor_copy(C0[:], y1_ps[:])
            pT0 = ps_b.tile([128, UC * BL], f32, tag="cmb", name="pT0")
            combine(C0, 0, UC, pT0, 0)
            nc.vector.tensor_add(y1pre[:], pT0[:], by1b_sb[:])
            nc.scalar.activation(y1t_sb[:], y1pre[:], AF.Tanh)

            # ---- 3) y = y1 @ Wy2 + by2 (col-tiled + bias-in-stream)
            y_ps = ps_a.tile([128, T], f32, tag="mm", name="y_ps")
            for r in range(2):
                for g4 in range(4):
                    k = 4 * r + g4
                    col_mm(y_ps, g4, y1t_sb[:, 8 * k : 8 * k + 8],
                           wy2_sb[k][:], start=(r == 0),
                           stop=(r == 1 and g4 > 0))
            # bias row into group 0 (counted once across the group sum)
            _mm(nc, y_ps[0:8, :], ones1_sb[:], by2row_sb[:],
                start=False, stop=True)
            C_y = c_pool.tile([128, T], bf16, tag="c", name="C_y")
            nc.vector.tensor_copy(C_y[:], y_ps[:])
            # row-major y for the output DMA
            y2_ps = ps_b.tile([BL, T], f32, tag="cmb", name="y2_ps")
            _mm(nc, y2_ps[:], E_sb[:], C_y[:], start=True, stop=True)
            nc.scalar.copy(y_sb[:], y2_ps[:])
            nc.sync.dma_start(d_out[:, ts(step_ap, T)], y_sb[:])
            # transposed y chunks -> xhy (gate lhsT)
            pxh = ps_b.tile([128, 4 * BL], f32, tag="cmb", name="pxh")
            combine(C_y, 0, 4, pxh, 0)
            nc.vector.tensor_copy(xhy_sb[:], pxh[:])

            drain_edots(2)

            # ---- 4) gates = x_h @ [Wi|Wf|Wo|Wg] (col-tiled), interleaved
            # with attention produce/e-dot; combine into gactT layout
            gT_ps = ps_g.tile([128, 4 * UC * BL], f32, tag="gT")
            for g in range(4):
                if g < 3:
                    produce(2 * g + 2)
                    produce(2 * g + 3)
                g_ps = ps_a.tile([128, U], f32, tag="mm", name="g_ps")
                for r in range(3):
                    for hh in range(2):
                        for g4 in range(4):
                            k = 4 * r + g4
                            lhsT = (xhy_sb[:, 8 * k : 8 * k + 8] if k < 4
                                    else rd[:, 8 * (k - 4) : 8 * (k - 4) + 8])
                            _mm(nc, g_ps[32 * g4 : 32 * g4 + 8, 512 * hh : 512 * (hh + 1)],
                                lhsT,
                                w4_sb[k][:, U * g + 512 * hh : U * g + 512 * (hh + 1)],
                                start=(r == 0), stop=(r == 2),
                                tile_position=(0, 32 * g4))
                C_g = c_pool.tile([128, U], bf16, tag="c", name="C_g")
                nc.vector.tensor_copy(C_g[:], g_ps[:])
                combine(C_g, 0, UC, gT_ps, 64 * g)
                drain_edots(min(2 * g + 2, UC))
            # bias + activations on the transposed gates
            nc.vector.tensor_add(gsum[:], gT_ps[:], b4b_sb[:])
            nc.scalar.activation(gact[:, 0 : 3 * 64], gsum[:, 0 : 3 * 64],
                                 AF.Sigmoid)
            nc.scalar.activation(gact[:, 3 * 64 : 4 * 64],
                                 gsum[:, 3 * 64 : 4 * 64], AF.Tanh)

            drain_edots(UC)

            # ---- 5) e = sigmoid(e_dot + be2); softmax via exp-poly
            C_e = c_pool.tile([128, S], bf16, tag="c", name="C_e")
            nc.vector.tensor_copy(C_e[:], e_ps[:])
            e2_ps = ps_b.tile([BL, S], f32, tag="cmb", name="e2_ps")
            _mm(nc, e2_ps[:], E_sb[:], C_e[:], start=True, stop=True)
            nc.scalar.activation(esig[:], e2_ps[:], AF.Sigmoid,
                                 bias=be2r_sb[:, 0:1])
            c0, c1, c2, c3, c4 = [float(c) for c in _EXP_C]
            nc.vector.tensor_scalar(er[:], esig[:], c4, c3, ALU.mult, ALU.add)
            nc.vector.tensor_mul(eq[:], er[:], esig[:])
            nc.vector.tensor_scalar(er[:], eq[:], 1.0, c2, ALU.mult, ALU.add)
            nc.vector.tensor_mul(eq[:], er[:], esig[:])
            nc.vector.tensor_scalar(er[:], eq[:], 1.0, c1, ALU.mult, ALU.add)
            nc.vector.tensor_mul(eq[:], er[:], esig[:])
            nc.vector.tensor_scalar(er[:], eq[:], 1.0, c0, ALU.mult, ALU.add)
            nc.vector.tensor_reduce(den[:], er[:], mybir.AxisListType.X, ALU.add)
            nc.vector.reciprocal(rden[:], den[:])
            # fold 1/den into a before bf16 cast -> context needs no rescale
            nc.vector.tensor_scalar_mul(eq[:], er[:], rden[:])
            nc.vector.tensor_copy(ea_bf[:], eq[:])
            psA = ps_b.tile([128, 16], bf16, tag="cmb", name="psA")
            for sc in range(2):
                nc.tensor.transpose(
                    psA[:, 8 * sc : 8 * sc + 8], ea_bf[:, 128 * sc : 128 * (sc + 1)],
                    id8[:])
                nc.vector.tensor_copy(
                    A_ld[:, 8 * sc : 8 * sc + 17 * 7 + 1 : 17], psA[:, 8 * sc : 8 * sc + 8])

            # ---- 6) context c = (A^T @ h) (col-tiled over ci), combine
            cx_ps = ps_a.tile([128, U], f32, tag="mm", name="cx_ps")
            for p in range(4):
                for hh in range(2):
                    for g4 in range(4):
                        ci = 4 * g4 + p
                        _mm(nc, cx_ps[32 * g4 : 32 * g4 + 8, 512 * hh : 512 * (hh + 1)],
                            A_ld[:, 8 * ci : 8 * ci + 8],
                            h_tiles[ci][:, 512 * hh : 512 * (hh + 1)],
                            start=(p == 0), stop=(p == 3),
                            tile_position=(0, 32 * g4))
            C_c = c_pool.tile([128, U], bf16, tag="c", name="C_c")
            nc.vector.tensor_copy(C_c[:], cx_ps[:])
            cT_ps = ps_b.tile([128, UC * BL], f32, tag="cmb", name="cT_ps")
            combine(C_c, 0, UC, cT_ps, 0)

            # ---- 7) LSTM cell in transposed [feat, b] layout
            gi = gact[:, 0:64]
            gf = gact[:, 64:128]
            go = gact[:, 128:192]
            gg = gact[:, 192:256]
            nc.vector.tensor_mul(t1f[:], gf, cT_ps[:])
            nc.vector.tensor_mul(t2f[:], gi, gg)
            nc.vector.tensor_add(cnew[:], t1f[:], t2f[:])
            nc.scalar.activation(thb[:], cnew[:], AF.Tanh)
            nc.vector.tensor_mul(wr[:], go, thb[:])

        # zero the PSUM pool buffers once: matmuls only ever write the four
        # 8-row col-group slices; the combine matmuls read all 128 rows, and
        # garbage (NaN) in unwritten rows would poison 0*NaN products.
        for _ in range(2):
            tz = ps_a.tile([128, U], f32, tag="mm", name="pzero")
            nc.vector.memset(tz[:], 0.0)
        tze = ps_e.tile([128, S], f32, tag="e", name="pzeroe")
        nc.vector.memset(tze[:], 0.0)

        assert nsteps % unroll == 0
        if static_loop:
            for it in range(nsteps // unroll):
                for j in range(unroll):
                    step_body(it * unroll + j, j)
        else:
            with tc.For_i(0, nsteps // unroll,
                  hint_engines=(mybir.EngineType.PE, mybir.EngineType.DVE,
                                mybir.EngineType.Activation)) as iv:
                base = nc.snap(iv * unroll)
                for j in range(unroll):
                    step_body(base + j, j)

    nc.finalize()
    return nc


# ---------------------------------------------------------------------------
# numpy-side input prep + SPMD execution

_NC_CACHE = {}
TRACE = False
TMPDIR = None
LAST_RESULTS = None


def _prep_shared(Wy1, by1, Wy2, by2, We1, be1, We2, be2, Wf, bfb, Wi, bi, Wg, bg,
                 Wo, bo):
    bf = ml_dtypes.bfloat16
    f = np.float32
    sh = {}
    Wsy = np.concatenate([Wy1, We1[U:]], axis=1)            # [1024, 2048]
    sh["Wsy"] = np.ascontiguousarray(Wsy.reshape(UC, 128, 2 * U)).astype(bf)
    sh["Wy2b"] = np.ascontiguousarray(Wy2.reshape(UC, 128, T)).astype(bf)
    W4 = np.concatenate([Wi, Wf, Wo, Wg], axis=1)           # [1536, 4096]
    sh["W4"] = np.ascontiguousarray(W4.reshape(TC4, 128, G)).astype(bf)
    sh["We1h"] = np.ascontiguousarray(We1[:U].reshape(UC, 128, U)).astype(bf)
    sh["We2c"] = np.ascontiguousarray(We2.reshape(UC, 128).T).astype(bf)
    E = np.zeros((128, BL), np.float32)
    for jj in range(4):
        for b in range(BL):
            E[32 * jj + b, b] = 1.0
    sh["Emat"] = E.astype(bf)
    sh["ones1"] = np.ones((1, BL), bf)
    sh["by2row"] = by2.reshape(1, T).astype(bf)
    # feature-transposed bias broadcasts: col = uc*8 + b
    by1T = by1.reshape(UC, 128).T                            # [128, UC]
    be1T = be1.reshape(UC, 128).T
    sh["by1b"] = np.repeat(by1T, BL, axis=1).astype(f)       # [128, UC*BL]
    sh["be1b"] = np.repeat(be1T, BL, axis=1).astype(f)
    b4 = np.concatenate([bi, bfb, bo, bg])                   # [4096]
    b4T = b4.reshape(4 * UC, 128).T                          # [128, 4*UC]
    sh["b4b"] = np.repeat(b4T, BL, axis=1).astype(bf)        # [128, 4*UC*BL]
    sh["be2r"] = np.full((BL, 1), float(be2[0]), f)
    return sh


def kernel(h, s_0, Wy1, by1, Wy2, by2, We1, be1, We2, be2,
           Wf, bf, Wi, bi, Wg, bg, Wo, bo, nsteps=S, unroll=8):
    h = np.asarray(h, np.float32)
    s_0 = np.asarray(s_0, np.float32)
    key = (nsteps, unroll)
    if key not in _NC_CACHE:
        _NC_CACHE[key] = build(nsteps=nsteps, unroll=unroll)
    nc = _NC_CACHE[key]

    sh = _prep_shared(Wy1, by1, Wy2, by2, We1, be1, We2, be2,
                      np.asarray(Wf), np.asarray(bf), np.asarray(Wi),
                      np.asarray(bi), np.asarray(Wg), np.asarray(bg),
                      np.asarray(Wo), np.asarray(bo))
    bfd = ml_dtypes.bfloat16
    in_maps = []
    for i in range(NCORES):
        hc = h[i * BL : (i + 1) * BL]                       # [8, 256, 1024]
        m = dict(sh)
        m["h_bf"] = np.ascontiguousarray(
            hc.reshape(BL, 2, 128, U).reshape(2 * BL, 128, U)).astype(bfd)
        m["hT_bf"] = np.ascontiguousarray(
            hc.transpose(2, 0, 1).reshape(UC, 128, BS)).astype(bfd)
        m["s0b"] = s_0[i * BL : (i + 1) * BL].astype(bfd)
        in_maps.append(m)

    res = run_bass_kernel_spmd(nc, in_maps, core_ids=list(range(NCORES)),
                               trace=TRACE, tmpdir=TMPDIR)
    global LAST_RESULTS
    LAST_RESULTS = res
    outs = [r["ys"].reshape(BL, S, T)[:, :nsteps, :] for r in res.results]
    full = np.concatenate(outs, axis=0)
    return full.astype(np.float32)


if __name__ == "__main__":
    print("building...")
    build(nsteps=4, unroll=4)
    print("build ok")
